# revision 25
# baseline (speedup 1.0000x reference)
"""DensePose FC head (4x fixed-offset deformable conv + relu) on 8 trn2 cores.

Sharding: pure data-parallel over the ROI dim N=128 -> 16 ROIs per core,
weights replicated. Each core runs the full 4-layer network out of SBUF:

  x1 = concat(features[16,512,28,28], maxpool4(fine_segm)[16,25,28,28])
  for w in w1..w4: x = relu(deform_conv_fixed(x, w))

The deformable conv with fixed integer taps is 9 shifted 1x1 GEMMs. We keep
every activation plane zero-padded to 34x34 in SBUF (pad 3 = max |tap|), so a
tap is just a shifted [128, 14, 28] access pattern and out-of-bounds reads hit
zeros, exactly matching the reference's zero-padding semantics. Matmuls are
bf16 (fp32 PSUM accumulation); each output chunk (half ROI = 392 positions,
one PSUM bank) accumulates taps x k-tiles in a single PSUM accumulation chain.
"""

import numpy as np
import ml_dtypes
from contextlib import ExitStack

import concourse.bass as bass
import concourse.tile as tile
from concourse import bacc
from concourse import mybir
from concourse.bass_utils import run_bass_kernel_spmd

TAPS = [(-1, -3), (-1, -1), (-1, 1), (0, 0), (0, 2), (-2, 1), (0, -1), (2, 0), (3, 1)]
N_CORES = 8
N_FULL = 128          # total ROIs
NLOC = N_FULL // N_CORES
CIN, P1, HID = 512, 25, 256
R, PAD, PR = 28, 3, 34       # spatial, pad, padded spatial
HALF = 14 * R                # 392 positions per chunk
HM = 112                     # fine_segm spatial
KTILES = [5, 2, 2, 2]        # k-tiles per layer (537 -> 4x128 feat + 1x pool(25))
RING = 3

f32 = mybir.dt.float32
bf16 = mybir.dt.bfloat16


def _w_tile_idx(ktiles, t, k):
    return t * ktiles + k


# tap (0,0) (full output coverage) first; see conv_layer
TAP_ORDER = [3, 0, 1, 2, 4, 5, 6, 7, 8]


# L1 pool-block cross-tap packing: the 25 pooled channels x 9 taps = 225 rows
# are packed into 2 dense k-tiles (rows (t, c) -> t*25+c), instead of paying a
# full 128-row matmul slot per tap for a 25-row contraction. L1 chains shrink
# from 45 to 38 matmuls.
W1_NTILES = 9 * 4 * 2 + 4      # 72 feature tiles + 2 packed k-tiles x 2 ochunks
N_W_TILES = [W1_NTILES, 36, 36, 36]


def _pool_tap_segments(t):
    """Packed-row segments for tap t: list of (ptile, row_lo, c_lo, n_rows)."""
    g0, g1 = t * 25, t * 25 + 25
    segs = []
    for p in range(2):
        lo, hi = max(g0, p * 128), min(g1, (p + 1) * 128)
        if lo < hi:
            segs.append((p, lo - p * 128, lo - g0, hi - lo))
    return segs


def build_nc(nloc=NLOC, finalize=True):
    nc = bacc.Bacc()
    feats = nc.dram_tensor("features", [nloc, CIN, R * R], f32, kind="ExternalInput")
    segm = nc.dram_tensor("fine_segm", [nloc, P1 * HM * HM], f32, kind="ExternalInput")
    wts = [
        nc.dram_tensor(f"w{l + 1}p", [128, N_W_TILES[l], 128], bf16,
                       kind="ExternalInput")
        for l in range(4)
    ]
    out = nc.dram_tensor("out", [nloc, HID, R * R], f32, kind="ExternalOutput")

    with tile.TileContext(nc) as tc, ExitStack() as ctx:
        _body(ctx, tc, nloc, feats, segm, wts, out)
    if finalize:
        nc.finalize()
    return nc


def _body(ctx, tc, nloc, feats, segm, wts, out):
    nc = tc.nc
    singles = ctx.enter_context(tc.tile_pool(name="singles", bufs=1))
    fstage = ctx.enter_context(tc.tile_pool(name="fstage", bufs=2))
    sstage = ctx.enter_context(tc.tile_pool(name="sstage", bufs=2))
    pmid = ctx.enter_context(tc.tile_pool(name="pmid", bufs=2))
    pstage = ctx.enter_context(tc.tile_pool(name="pstage", bufs=2))
    ostage = ctx.enter_context(tc.tile_pool(name="ostage", bufs=8))
    psum = ctx.enter_context(tc.tile_pool(name="psum", bufs=8, space="PSUM"))

    # resident weights: one SBUF tile per (layer, output-channel chunk) so the
    # first chains (o=0) only wait on half the weight bytes; each tile DMAs in
    # two chunks on separate HW queues
    w_sb = []
    for l in range(4):
        half = N_W_TILES[l] // 2
        per_o = []
        for o in range(2):
            wt = singles.tile([128, half, 128], bf16, tag=f"w{l}o{o}",
                              name=f"w{l}o{o}")
            for q in range(2):
                lo, hi = half * q // 2, half * (q + 1) // 2
                nc.sync.dma_start(out=wt[:, lo:hi, :],
                                  in_=wts[l][:, o * half + lo:o * half + hi, :])
            per_o.append(wt)
        w_sb.append(per_o)

    # padded activation rings: xf[4 feature blocks], xp (pooled block),
    # y[l][2 out blocks] for layers 1..3 outputs
    def padded_tile(tag):
        return singles.tile([128, PR, PR], bf16, tag=tag, name=tag)

    xf = [[padded_tile(f"xf{b}_{i}") for i in range(RING)] for b in range(4)]
    xp = [padded_tile(f"xp_{i}") for i in range(RING)]
    y = [[[padded_tile(f"y{l}_{b}_{i}") for i in range(RING)] for b in range(2)]
         for l in range(3)]
    # cross-tap packed pooled-block rhs tiles (rows = (tap, channel))
    pk = [[singles.tile([128, R, R], bf16, tag=f"pk{p}_{i}", name=f"pk{p}_{i}")
           for i in range(RING)] for p in range(2)]

    # one-time zeroing of pad borders (and unused partitions of the pooled
    # block); interiors get fully rewritten every use, pads stay zero.
    ms_engines = [nc.gpsimd]
    n_ms = 0

    def memset0(ap):
        nonlocal n_ms
        ms_engines[n_ms % len(ms_engines)].memset(ap, 0.0)
        n_ms += 1

    # xf/y borders are never read (clipped tap windows stay in the interior),
    # so only xp (whose pads feed the packed-pool shifted copies) and the pk
    # tail rows need one-time zeroing.
    for t in xp:
        # full-tile zero: covers pads AND the unused channel partitions
        # (25..127), which compute engines can't address as a base partition
        memset0(t[:, :, :])
    for pt in pk:
        for t in pt:
            # rows 0..224 fully rewritten every ROI; rows 225.. stay zero
            memset0(t[:, :, :])

    def produce_l1_input(r):
        slot = r % RING
        # features: DRAM fp32 -> staging -> bf16 padded tiles. One DMA per
        # 128-channel block: a single big DMA lands on ONE of the 8 HW queues
        # (~11 GB/s each for these 3KB lines) and can't keep up with the
        # pipeline; four DMAs spread across queues.
        for b in range(4):
            fst = fstage.tile([128, R * R], f32, tag=f"fst{b}", name=f"fst{b}")
            for q in range(2):
                nc.sync.dma_start(
                    out=fst[64 * q:64 * (q + 1), :],
                    in_=feats[r, b * 128 + 64 * q:b * 128 + 64 * (q + 1), :])
            src = fst.rearrange("p (h w) -> p h w", w=R)
            dst = xf[b][slot][:, PAD:PAD + R, PAD:PAD + R]
            if b % 2 == 0:
                nc.vector.tensor_copy(out=dst, in_=src)
            else:
                nc.gpsimd.tensor_copy(out=dst, in_=src)
        # fine_segm: DRAM fp32 -> [100, 3136] staging -> 4x4 maxpool (one XY
        # reduce; partition p = 25 channels x 4 row-quarters) -> bf16 -> xp
        sst = sstage.tile([128, 4 * 28 * 28], f32, name="sst")
        segm_r = segm[r].rearrange("(p x) -> p x", x=3136)
        for b in range(4):
            nc.sync.dma_start(out=sst[25 * b:25 * (b + 1)],
                              in_=segm_r[25 * b:25 * (b + 1)])
        pooled = pmid.tile([128, 196], bf16, name="pooled")
        nc.vector.tensor_reduce(
            out=pooled[:100],
            in_=sst[:100].rearrange("p (h0 h1 w0 w1) -> p h0 w0 h1 w1",
                                    h0=7, h1=4, w0=28, w1=4),
            axis=mybir.AxisListType.XY,
            op=mybir.AluOpType.max,
        )
        # [100, 196] -> [25, 28, 28] interior of the padded pool block
        dst = xp[slot][:P1, PAD:PAD + R, PAD:PAD + R].rearrange(
            "p (q h) w -> p q h w", q=4)
        nc.sync.dma_start(out=dst, in_=pooled[:100])
        # scatter 9 shifted copies of the pooled plane into packed STAGING
        # tiles (SBUF->SBUF DMA; full 28x28 shifted windows read pad zeros),
        # then one DMA per packed tile. The funnel keeps the consuming
        # matmuls at a single DMA-queue wait (walrus rejects matmuls with
        # many sync waits; the scatter DMAs fan out across all 8 queues).
        pks = [pstage.tile([128, R, R], bf16, tag=f"pks{p}", name=f"pks{p}")
               for p in range(2)]
        for t, (dh, dw) in enumerate(TAPS):
            src_plane = xp[slot][:P1, PAD + dh:PAD + dh + R, PAD + dw:PAD + dw + R]
            for p, row_lo, c_lo, n in _pool_tap_segments(t):
                nc.sync.dma_start(
                    out=pks[p][row_lo:row_lo + n, :, :],
                    in_=src_plane[c_lo:c_lo + n],
                )
        nc.sync.dma_start(out=pk[0][slot][:, :, :], in_=pks[0][:, :, :])
        nc.sync.dma_start(out=pk[1][slot][:97, :, :], in_=pks[1][:97, :, :])

    def conv_layer(r, l):
        slot = r % RING
        if l == 0:
            xin = [xf[b][slot] for b in range(4)]
            ktiles, n_mm = 4, 9 * 4 + 2
        else:
            xin = [y[l - 1][b][slot] for b in range(2)]
            ktiles, n_mm = 2, 9 * 2
        for hf in range(2):
            for o in range(2):
                ps = psum.tile([128, 14, R], f32, name="ps")
                idx = 0
                # taps are clipped to their valid output window (the border
                # contributions are zero) - skips ~8% of PE cycles. Tap (0,0)
                # goes first: it covers the full chunk, so start=True zeroes
                # every element before the partial-window taps accumulate.
                for t in TAP_ORDER:
                    dh, dw = TAPS[t]
                    h_lo = max(max(0, -dh), 14 * hf)
                    h_hi = min(min(R, R - dh), 14 * hf + 14)
                    w_lo = max(0, -dw)
                    w_hi = min(R, R - dw)
                    out_ap = ps[:, h_lo - 14 * hf:h_hi - 14 * hf, w_lo:w_hi]
                    for k in range(ktiles):
                        nc.tensor.matmul(
                            out_ap,
                            lhsT=w_sb[l][o][:, _w_tile_idx(ktiles, t, k), :],
                            rhs=xin[k][:, PAD + h_lo + dh:PAD + h_hi + dh,
                                       PAD + w_lo + dw:PAD + w_hi + dw],
                            start=(idx == 0),
                            stop=(idx == n_mm - 1),
                        )
                        idx += 1
                if l == 0:
                    for p in range(2):
                        nc.tensor.matmul(
                            ps,
                            lhsT=w_sb[0][o][:, 36 + p, :],
                            rhs=pk[p][slot][:, hf * 14:hf * 14 + 14, :],
                            start=False,
                            stop=(idx == n_mm - 1),
                        )
                        idx += 1
                if l < 3:
                    dst = y[l][o][slot][:, PAD + 14 * hf:PAD + 14 * (hf + 1),
                                        PAD:PAD + R]
                    if (hf + o) % 2 == 0:
                        nc.scalar.activation(out=dst, in_=ps,
                                             func=mybir.ActivationFunctionType.Relu)
                    else:
                        nc.vector.tensor_scalar_max(out=dst, in0=ps, scalar1=0.0)
                else:
                    ot = ostage.tile([128, 14, R], f32, name="ot")
                    if (hf + o) % 2 == 0:
                        nc.scalar.activation(out=ot, in_=ps,
                                             func=mybir.ActivationFunctionType.Relu)
                    else:
                        nc.vector.tensor_scalar_max(out=ot, in0=ps, scalar1=0.0)
                    nc.sync.dma_start(
                        out=out[r, o * 128:(o + 1) * 128,
                                hf * HALF:(hf + 1) * HALF],
                        in_=ot.rearrange("p h w -> p (h w)"),
                    )

    # software-pipelined staircase over ROIs: at step s emit input-produce for
    # ROI s+2 (two steps of lead so the staging DMAs never gate PE) and layer
    # l for ROI s-l. Ring depth 3 still works: xf[r%3] is written at step r-2
    # and last read at step r.
    produce_l1_input(0)
    produce_l1_input(1)
    for s in range(nloc + 3):
        if s + 2 < nloc:
            produce_l1_input(s + 2)
        for l in range(4):
            r = s - l
            if 0 <= r < nloc:
                conv_layer(r, l)


def _pack_weights(w, ktiles):
    """[O=256, Cin, 3, 3] fp32 -> [128, 9*ktiles*2, 128] bf16 lhsT tiles,
    o-major: tile (o, t, k) at index o*(9*ktiles) + t*ktiles + k."""
    O, Cin = w.shape[:2]
    wf = w.reshape(O, Cin, 9)
    half = 9 * ktiles
    wp = np.zeros((128, 2 * half, 128), np.float32)
    for t in range(9):
        for k in range(ktiles):
            cs = min(128, Cin - k * 128)
            for o in range(2):
                ti = o * half + _w_tile_idx(ktiles, t, k)
                blk = wf[o * 128:(o + 1) * 128, k * 128:k * 128 + cs, t]
                wp[:cs, ti, :] = blk.T
    return wp.astype(ml_dtypes.bfloat16)


def _pack_w1(w):
    """w1 [256, 537, 3, 3] -> [128, 76, 128], o-major: o-block of 38 tiles =
    36 feature lhsT tiles (t*4+k) then 2 cross-tap-packed pooled-block tiles
    (packed row t*25+c <-> tap t, ch c)."""
    wf = w.reshape(HID, 537, 9)
    wp = np.zeros((128, W1_NTILES, 128), np.float32)
    half = 38
    for t in range(9):
        for k in range(4):
            for o in range(2):
                ti = o * half + _w_tile_idx(4, t, k)
                blk = wf[o * 128:(o + 1) * 128, k * 128:(k + 1) * 128, t]
                wp[:, ti, :] = blk.T
    for p in range(2):
        for o in range(2):
            ti = o * half + 36 + p
            for row in range(128):
                g = p * 128 + row
                if g >= 225:
                    break
                t, c = divmod(g, 25)
                wp[row, ti, :] = wf[o * 128:(o + 1) * 128, CIN + c, t]
    return wp.astype(ml_dtypes.bfloat16)


_CACHE = {}


def kernel(features, fine_segm, w1, w2, w3, w4):
    assert features.shape == (N_FULL, CIN, R, R), features.shape
    assert fine_segm.shape == (N_FULL, P1, HM, HM), fine_segm.shape

    if "nc" not in _CACHE:
        _CACHE["nc"] = build_nc(NLOC)
    nc = _CACHE["nc"]

    wpacked = {"w1p": _pack_w1(np.asarray(w1, np.float32))}
    for l, w in enumerate([w2, w3, w4], start=1):
        wpacked[f"w{l + 1}p"] = _pack_weights(np.asarray(w, np.float32), 2)
    featsr = np.ascontiguousarray(np.asarray(features, np.float32)
                                  .reshape(N_FULL, CIN, R * R))
    segmr = np.ascontiguousarray(np.asarray(fine_segm, np.float32)
                                 .reshape(N_FULL, P1 * HM * HM))

    in_maps = []
    for c in range(N_CORES):
        sl = slice(c * NLOC, (c + 1) * NLOC)
        in_maps.append({"features": featsr[sl], "fine_segm": segmr[sl], **wpacked})

    res = run_bass_kernel_spmd(nc, in_maps, list(range(N_CORES)))
    outs = [res.results[c]["out"].reshape(NLOC, HID, R, R) for c in range(N_CORES)]
    return np.concatenate(outs, axis=0).astype(np.float32)


# revision 26
# speedup vs baseline: 1.0393x; 1.0393x over previous
"""DensePose FC head (4x fixed-offset deformable conv + relu) on 8 trn2 cores.

Sharding: pure data-parallel over the ROI dim N=128 -> 16 ROIs per core,
weights replicated. Each core runs the full 4-layer network out of SBUF:

  x1 = concat(features[16,512,28,28], maxpool4(fine_segm)[16,25,28,28])
  for w in w1..w4: x = relu(deform_conv_fixed(x, w))

The deformable conv with fixed integer taps is 9 shifted 1x1 GEMMs. We keep
every activation plane zero-padded to 34x34 in SBUF (pad 3 = max |tap|), so a
tap is just a shifted [128, 14, 28] access pattern and out-of-bounds reads hit
zeros, exactly matching the reference's zero-padding semantics. Matmuls are
bf16 (fp32 PSUM accumulation); each output chunk (half ROI = 392 positions,
one PSUM bank) accumulates taps x k-tiles in a single PSUM accumulation chain.
"""

import numpy as np
import ml_dtypes
from contextlib import ExitStack

import concourse.bass as bass
import concourse.tile as tile
from concourse import bacc
from concourse import mybir
from concourse.bass_utils import run_bass_kernel_spmd

TAPS = [(-1, -3), (-1, -1), (-1, 1), (0, 0), (0, 2), (-2, 1), (0, -1), (2, 0), (3, 1)]
N_CORES = 8
N_FULL = 128          # total ROIs
NLOC = N_FULL // N_CORES
CIN, P1, HID = 512, 25, 256
R, PAD, PR = 28, 3, 34       # spatial, pad, padded spatial
HALF = 14 * R                # 392 positions per chunk
HM = 112                     # fine_segm spatial
KTILES = [5, 2, 2, 2]        # k-tiles per layer (537 -> 4x128 feat + 1x pool(25))
RING = 3

f32 = mybir.dt.float32
bf16 = mybir.dt.bfloat16


def _w_tile_idx(ktiles, t, k, o):
    return (t * ktiles + k) * 2 + o


# tap (0,0) (full output coverage) first; see conv_layer
TAP_ORDER = [3, 0, 1, 2, 4, 5, 6, 7, 8]


# L1 pool-block cross-tap packing: the 25 pooled channels x 9 taps = 225 rows
# are packed into 2 dense k-tiles (rows (t, c) -> t*25+c), instead of paying a
# full 128-row matmul slot per tap for a 25-row contraction. L1 chains shrink
# from 45 to 38 matmuls.
W1_NTILES = 9 * 4 * 2 + 4      # 72 feature tiles + 2 packed k-tiles x 2 ochunks
N_W_TILES = [W1_NTILES, 36, 36, 36]


def _pool_tap_segments(t):
    """Packed-row segments for tap t: list of (ptile, row_lo, c_lo, n_rows)."""
    g0, g1 = t * 25, t * 25 + 25
    segs = []
    for p in range(2):
        lo, hi = max(g0, p * 128), min(g1, (p + 1) * 128)
        if lo < hi:
            segs.append((p, lo - p * 128, lo - g0, hi - lo))
    return segs


def build_nc(nloc=NLOC, finalize=True):
    nc = bacc.Bacc()
    feats = nc.dram_tensor("features", [nloc, CIN, R * R], f32, kind="ExternalInput")
    segm = nc.dram_tensor("fine_segm", [nloc, P1 * HM * HM], f32, kind="ExternalInput")
    wts = [
        nc.dram_tensor(f"w{l + 1}p", [128, N_W_TILES[l], 128], bf16,
                       kind="ExternalInput")
        for l in range(4)
    ]
    out = nc.dram_tensor("out", [nloc, HID, R * R], f32, kind="ExternalOutput")

    with tile.TileContext(nc) as tc, ExitStack() as ctx:
        _body(ctx, tc, nloc, feats, segm, wts, out)
    if finalize:
        nc.finalize()
    return nc


def _body(ctx, tc, nloc, feats, segm, wts, out):
    nc = tc.nc
    singles = ctx.enter_context(tc.tile_pool(name="singles", bufs=1))
    fstage = ctx.enter_context(tc.tile_pool(name="fstage", bufs=2))
    sstage = ctx.enter_context(tc.tile_pool(name="sstage", bufs=2))
    pmid = ctx.enter_context(tc.tile_pool(name="pmid", bufs=2))
    pstage = ctx.enter_context(tc.tile_pool(name="pstage", bufs=2))
    ostage = ctx.enter_context(tc.tile_pool(name="ostage", bufs=8))
    psum = ctx.enter_context(tc.tile_pool(name="psum", bufs=8, space="PSUM"))

    # resident weights, one [128, T, 128] tile per layer; DMA in four chunks
    # per layer so the loads spread across HW DMA queues
    w_sb = []
    for l in range(4):
        wt = singles.tile([128, N_W_TILES[l], 128], bf16, tag=f"w{l}", name=f"w{l}")
        ntl = N_W_TILES[l]
        bounds = [ntl * i // 4 for i in range(5)]
        for q in range(4):
            lo, hi = bounds[q], bounds[q + 1]
            nc.sync.dma_start(out=wt[:, lo:hi, :], in_=wts[l][:, lo:hi, :])
        w_sb.append(wt)

    # padded activation rings: xf[4 feature blocks], xp (pooled block),
    # y[l][2 out blocks] for layers 1..3 outputs
    def padded_tile(tag):
        return singles.tile([128, PR, PR], bf16, tag=tag, name=tag)

    xf = [[padded_tile(f"xf{b}_{i}") for i in range(RING)] for b in range(4)]
    xp = [padded_tile(f"xp_{i}") for i in range(RING)]
    y = [[[padded_tile(f"y{l}_{b}_{i}") for i in range(RING)] for b in range(2)]
         for l in range(3)]
    # cross-tap packed pooled-block rhs tiles (rows = (tap, channel))
    pk = [[singles.tile([128, R, R], bf16, tag=f"pk{p}_{i}", name=f"pk{p}_{i}")
           for i in range(RING)] for p in range(2)]

    # one-time zeroing of pad borders (and unused partitions of the pooled
    # block); interiors get fully rewritten every use, pads stay zero.
    ms_engines = [nc.gpsimd]
    n_ms = 0

    def memset0(ap):
        nonlocal n_ms
        ms_engines[n_ms % len(ms_engines)].memset(ap, 0.0)
        n_ms += 1

    # xf/y borders are never read (clipped tap windows stay in the interior),
    # so only xp (whose pads feed the packed-pool shifted copies) and the pk
    # tail rows need one-time zeroing.
    for t in xp:
        # full-tile zero: covers pads AND the unused channel partitions
        # (25..127), which compute engines can't address as a base partition
        memset0(t[:, :, :])
    for pt in pk:
        for t in pt:
            # rows 0..224 fully rewritten every ROI; rows 225.. stay zero
            memset0(t[:, :, :])

    def produce_l1_input(r):
        slot = r % RING
        # features: DRAM fp32 -> staging -> bf16 padded tiles. One DMA per
        # 128-channel block: a single big DMA lands on ONE of the 8 HW queues
        # (~11 GB/s each for these 3KB lines) and can't keep up with the
        # pipeline; four DMAs spread across queues.
        fst = fstage.tile([128, 4, R * R], f32, name="fst")
        for b in range(4):
            nc.sync.dma_start(out=fst[:, b, :],
                              in_=feats[r, b * 128:(b + 1) * 128, :])
        for b in range(4):
            src = fst[:, b, :].rearrange("p (h w) -> p h w", w=R)
            dst = xf[b][slot][:, PAD:PAD + R, PAD:PAD + R]
            if b % 2 == 0:
                nc.vector.tensor_copy(out=dst, in_=src)
            else:
                nc.gpsimd.tensor_copy(out=dst, in_=src)
        # fine_segm: DRAM fp32 -> [100, 3136] staging -> 4x4 maxpool (one XY
        # reduce; partition p = 25 channels x 4 row-quarters) -> bf16 -> xp
        sst = sstage.tile([128, 4 * 28 * 28], f32, name="sst")
        segm_r = segm[r].rearrange("(p x) -> p x", x=3136)
        for b in range(4):
            nc.sync.dma_start(out=sst[25 * b:25 * (b + 1)],
                              in_=segm_r[25 * b:25 * (b + 1)])
        pooled = pmid.tile([128, 196], bf16, name="pooled")
        nc.vector.tensor_reduce(
            out=pooled[:100],
            in_=sst[:100].rearrange("p (h0 h1 w0 w1) -> p h0 w0 h1 w1",
                                    h0=7, h1=4, w0=28, w1=4),
            axis=mybir.AxisListType.XY,
            op=mybir.AluOpType.max,
        )
        # [100, 196] -> [25, 28, 28] interior of the padded pool block
        dst = xp[slot][:P1, PAD:PAD + R, PAD:PAD + R].rearrange(
            "p (q h) w -> p q h w", q=4)
        nc.sync.dma_start(out=dst, in_=pooled[:100])
        # scatter 9 shifted copies of the pooled plane into packed STAGING
        # tiles (SBUF->SBUF DMA; full 28x28 shifted windows read pad zeros),
        # then one DMA per packed tile. The funnel keeps the consuming
        # matmuls at a single DMA-queue wait (walrus rejects matmuls with
        # many sync waits; the scatter DMAs fan out across all 8 queues).
        pks = [pstage.tile([128, R, R], bf16, tag=f"pks{p}", name=f"pks{p}")
               for p in range(2)]
        for t, (dh, dw) in enumerate(TAPS):
            src_plane = xp[slot][:P1, PAD + dh:PAD + dh + R, PAD + dw:PAD + dw + R]
            for p, row_lo, c_lo, n in _pool_tap_segments(t):
                nc.sync.dma_start(
                    out=pks[p][row_lo:row_lo + n, :, :],
                    in_=src_plane[c_lo:c_lo + n],
                )
        nc.sync.dma_start(out=pk[0][slot][:, :, :], in_=pks[0][:, :, :])
        nc.sync.dma_start(out=pk[1][slot][:97, :, :], in_=pks[1][:97, :, :])

    def conv_layer(r, l):
        slot = r % RING
        if l == 0:
            xin = [xf[b][slot] for b in range(4)]
            ktiles, n_mm = 4, 9 * 4 + 2
        else:
            xin = [y[l - 1][b][slot] for b in range(2)]
            ktiles, n_mm = 2, 9 * 2
        for hf in range(2):
            for o in range(2):
                ps = psum.tile([128, 14, R], f32, name="ps")
                idx = 0
                # taps are clipped to their valid output window (the border
                # contributions are zero) - skips ~8% of PE cycles. Tap (0,0)
                # goes first: it covers the full chunk, so start=True zeroes
                # every element before the partial-window taps accumulate.
                for t in TAP_ORDER:
                    dh, dw = TAPS[t]
                    h_lo = max(max(0, -dh), 14 * hf)
                    h_hi = min(min(R, R - dh), 14 * hf + 14)
                    w_lo = max(0, -dw)
                    w_hi = min(R, R - dw)
                    out_ap = ps[:, h_lo - 14 * hf:h_hi - 14 * hf, w_lo:w_hi]
                    for k in range(ktiles):
                        nc.tensor.matmul(
                            out_ap,
                            lhsT=w_sb[l][:, _w_tile_idx(ktiles, t, k, o), :],
                            rhs=xin[k][:, PAD + h_lo + dh:PAD + h_hi + dh,
                                       PAD + w_lo + dw:PAD + w_hi + dw],
                            start=(idx == 0),
                            stop=(idx == n_mm - 1),
                        )
                        idx += 1
                if l == 0:
                    for p in range(2):
                        nc.tensor.matmul(
                            ps,
                            lhsT=w_sb[0][:, 72 + p * 2 + o, :],
                            rhs=pk[p][slot][:, hf * 14:hf * 14 + 14, :],
                            start=False,
                            stop=(idx == n_mm - 1),
                        )
                        idx += 1
                if l < 3:
                    dst = y[l][o][slot][:, PAD + 14 * hf:PAD + 14 * (hf + 1),
                                        PAD:PAD + R]
                    if (hf + o) % 2 == 0:
                        nc.scalar.activation(out=dst, in_=ps,
                                             func=mybir.ActivationFunctionType.Relu)
                    else:
                        nc.vector.tensor_scalar_max(out=dst, in0=ps, scalar1=0.0)
                else:
                    ot = ostage.tile([128, 14, R], f32, name="ot")
                    if (hf + o) % 2 == 0:
                        nc.scalar.activation(out=ot, in_=ps,
                                             func=mybir.ActivationFunctionType.Relu)
                    else:
                        nc.vector.tensor_scalar_max(out=ot, in0=ps, scalar1=0.0)
                    nc.sync.dma_start(
                        out=out[r, o * 128:(o + 1) * 128,
                                hf * HALF:(hf + 1) * HALF],
                        in_=ot.rearrange("p h w -> p (h w)"),
                    )

    # software-pipelined staircase over ROIs: at step s emit input-produce for
    # ROI s+2 (two steps of lead so the staging DMAs never gate PE) and layer
    # l for ROI s-l. Ring depth 3 still works: xf[r%3] is written at step r-2
    # and last read at step r.
    produce_l1_input(0)
    produce_l1_input(1)
    for s in range(nloc + 3):
        if s + 2 < nloc:
            produce_l1_input(s + 2)
        for l in range(4):
            r = s - l
            if 0 <= r < nloc:
                conv_layer(r, l)


def _pack_weights(w, ktiles):
    """[O=256, Cin, 3, 3] fp32 -> [128, 9*ktiles*2, 128] bf16 lhsT tiles."""
    O, Cin = w.shape[:2]
    wf = w.reshape(O, Cin, 9)
    wp = np.zeros((128, 9 * ktiles * 2, 128), np.float32)
    for t in range(9):
        for k in range(ktiles):
            cs = min(128, Cin - k * 128)
            for o in range(2):
                ti = _w_tile_idx(ktiles, t, k, o)
                blk = wf[o * 128:(o + 1) * 128, k * 128:k * 128 + cs, t]
                wp[:cs, ti, :] = blk.T
    return wp.astype(ml_dtypes.bfloat16)


def _pack_w1(w):
    """w1 [256, 537, 3, 3] -> [128, 76, 128]: 72 feature lhsT tiles plus 4
    cross-tap-packed pooled-block tiles (packed row t*25+c <-> tap t, ch c)."""
    wf = w.reshape(HID, 537, 9)
    wp = np.zeros((128, W1_NTILES, 128), np.float32)
    for t in range(9):
        for k in range(4):
            for o in range(2):
                ti = _w_tile_idx(4, t, k, o)
                blk = wf[o * 128:(o + 1) * 128, k * 128:(k + 1) * 128, t]
                wp[:, ti, :] = blk.T
    for p in range(2):
        for o in range(2):
            ti = 72 + p * 2 + o
            for row in range(128):
                g = p * 128 + row
                if g >= 225:
                    break
                t, c = divmod(g, 25)
                wp[row, ti, :] = wf[o * 128:(o + 1) * 128, CIN + c, t]
    return wp.astype(ml_dtypes.bfloat16)


_CACHE = {}


def kernel(features, fine_segm, w1, w2, w3, w4):
    assert features.shape == (N_FULL, CIN, R, R), features.shape
    assert fine_segm.shape == (N_FULL, P1, HM, HM), fine_segm.shape

    if "nc" not in _CACHE:
        _CACHE["nc"] = build_nc(NLOC)
    nc = _CACHE["nc"]

    wpacked = {"w1p": _pack_w1(np.asarray(w1, np.float32))}
    for l, w in enumerate([w2, w3, w4], start=1):
        wpacked[f"w{l + 1}p"] = _pack_weights(np.asarray(w, np.float32), 2)
    featsr = np.ascontiguousarray(np.asarray(features, np.float32)
                                  .reshape(N_FULL, CIN, R * R))
    segmr = np.ascontiguousarray(np.asarray(fine_segm, np.float32)
                                 .reshape(N_FULL, P1 * HM * HM))

    in_maps = []
    for c in range(N_CORES):
        sl = slice(c * NLOC, (c + 1) * NLOC)
        in_maps.append({"features": featsr[sl], "fine_segm": segmr[sl], **wpacked})

    res = run_bass_kernel_spmd(nc, in_maps, list(range(N_CORES)))
    outs = [res.results[c]["out"].reshape(NLOC, HID, R, R) for c in range(N_CORES)]
    return np.concatenate(outs, axis=0).astype(np.float32)
